# revision 1
# baseline (speedup 1.0000x reference)
"""GroupQueryAttention on 8 trn2 cores.

Sharding: core c = (b, g) with b = c // 4 (batch), g = c % 4 (KV group).
Each core computes the 4 query heads of its group against its batch's
sequence, plus the row-slice of the output projection for those heads.
Host sums the 4 partial outputs per batch (row-parallel Wo) and adds bo.

Per-core layout strategy (everything "transposed", partition dim = the
contraction dim of the next matmul):
  xT   [e=128 x 8, s=2048]   via PE transpose of DMA'd x tiles
  qT   [d=256, s=2048]       = Wq_g^T x^T   (+bq, per-partition add)
  kvT  [d=128, s=2048]       rows 0:64 = k^T, 64:128 = v^T (+bk/bv)
  v_aug[t=128 x 16, 65]      v re-transposed, col 64 = 1.0 (Z column)
  per (head, s-half): for t in 16 tiles:
      scoresT psum [t=128, s=1024] = k_h^T(tile)^T @ q_h^T
      E = exp(0.125 * scoresT)  (ACT, PSUM -> SBUF)
      A@V psum [65, s=1024] += v_aug(t)^T @ E    (row 64 accumulates Z)
  normalize: U^T / Z via reciprocal + PE broadcast of 1/Z over 64 rows
  out^T psum [e=128, s=512] = Wo_g^T slice @ U^T  -> DMA to DRAM [E, S]
"""

import os
import numpy as np
from contextlib import ExitStack

import concourse.bass as bass
import concourse.bacc as bacc
import concourse.mybir as mybir
from concourse.tile import TileContext
from concourse.bass_utils import run_bass_kernel_spmd
from concourse.masks import make_identity

B, S, E = 2, 2048, 1024
H, G, HD = 16, 4, 64
GH = H // G          # heads per group = 4
DG = GH * HD         # q cols per group = 256
N_CORES = 8

FP = mybir.dt.float32
# float32r streams 1 row/cycle (vs 4 for plain fp32) when N >= 256.
MM_FAST = os.environ.get("GQA_MM_FP32R", "1") == "1"
MM_DT = mybir.dt.float32r if MM_FAST else mybir.dt.float32

KE = E // 128        # 8 contraction chunks for projections
NT = S // 128        # 16 t tiles
SC = 512             # matmul moving-dim chunk
NSC = S // SC        # 4
SH = 1024            # s-half for attention psum accumulators
NSH = S // SH        # 2


def mm(x):
    """bitcast an AP for the tensor engine's fast fp32 path"""
    return x.bitcast(MM_DT) if MM_FAST else x


def build_program() -> bass.Bass:
    # Bacc (not plain Bass): its compile() runs move_matmul_waits_to_ldweights
    # + generate_event_semaphores, without which walrus rejects matmuls that
    # accumulated >1 semaphore wait ("Too many sync wait commands").
    nc = bacc.Bacc(None, target_bir_lowering=False)
    x = nc.dram_tensor("xc", [S, E], FP, kind="ExternalInput")
    wq = nc.dram_tensor("wq", [E, DG], FP, kind="ExternalInput")
    wkv = nc.dram_tensor("wkv", [E, 2 * HD], FP, kind="ExternalInput")
    wo = nc.dram_tensor("wo", [DG, E], FP, kind="ExternalInput")
    bq = nc.dram_tensor("bq", [DG], FP, kind="ExternalInput")
    bkv = nc.dram_tensor("bkv", [2 * HD], FP, kind="ExternalInput")
    ot = nc.dram_tensor("ot", [E, S], FP, kind="ExternalOutput")

    with TileContext(nc) as tc, ExitStack() as ctx:
        const = ctx.enter_context(tc.tile_pool(name="const", bufs=1))
        xload = ctx.enter_context(tc.tile_pool(name="xload", bufs=2))
        big = ctx.enter_context(tc.tile_pool(name="big", bufs=1))
        # PSUM: rot(2 banks) + psc(4 banks) + pav(2 banks) = 8 banks
        rot = ctx.enter_context(tc.tile_pool(name="rot", bufs=2, space="PSUM"))
        pscp = ctx.enter_context(tc.tile_pool(name="pscp", bufs=2, space="PSUM"))
        pavp = ctx.enter_context(tc.tile_pool(name="pavp", bufs=1, space="PSUM"))

        # ---- constants ----
        ident = const.tile([128, 128], FP)
        make_identity(nc, ident)
        # memset cannot emit fp32r (ISA check): memset fp32 scratch, then
        # round through a DVE copy into the matmul-facing ones tiles.
        ones_f = const.tile([128, HD], FP)
        nc.vector.memset(ones_f, 1.0)
        ones_col = const.tile([128, HD], FP)
        nc.vector.tensor_copy(out=mm(ones_col), in_=ones_f)

        # fp32r matmul operands must be written pre-rounded by their
        # producing instruction (BIR verifier rule), and a DMA cannot round:
        # stage each weight load through a scratch tile, rounding via DVE.
        wq_sb = const.tile([128, KE, DG], FP)
        wkv_sb = const.tile([128, KE, 2 * HD], FP)
        wo_sb = const.tile([64, GH, E], FP)
        wq_r = wq.rearrange("(j p) c -> p j c", p=128)
        wo_r = wo.rearrange("(c p) e -> p c e", p=64)
        for dst, src_ap in (
            (wq_sb[:, 0:4, :], wq_r[:, 0:4, :]),
            (wq_sb[:, 4:8, :], wq_r[:, 4:8, :]),
            (wkv_sb[:, :, :], wkv.rearrange("(j p) c -> p j c", p=128)),
            (wo_sb[:, 0:1, :], wo_r[:, 0:1, :]),
            (wo_sb[:, 1:2, :], wo_r[:, 1:2, :]),
            (wo_sb[:, 2:3, :], wo_r[:, 2:3, :]),
            (wo_sb[:, 3:4, :], wo_r[:, 3:4, :]),
        ):
            pdim = dst.shape[0]
            wtmp = xload.tile([128, E], FP, tag="x_sb")
            wview = wtmp[0:pdim, :].rearrange("p (a b) -> p a b", b=dst.shape[-1])
            wview = wview[:, 0 : dst.shape[1], :]
            nc.sync.dma_start(out=wview, in_=src_ap)
            nc.vector.tensor_copy(out=mm(dst), in_=wview)
        bq_sb = const.tile([64, GH], FP)
        nc.sync.dma_start(out=bq_sb, in_=bq.rearrange("(j p) -> p j", p=64))
        bkv_sb = const.tile([128, 1], FP)
        nc.sync.dma_start(out=bkv_sb, in_=bkv.rearrange("(j p) -> p j", p=128))

        # ---- persistent activations ----
        # xT is only needed through phase 2; its own pool is closed after the
        # projections so attention-phase pools reuse its 64 KB/partition.
        xtp_cm = tc.tile_pool(name="xtp", bufs=1)
        xtp = xtp_cm.__enter__()
        xT = xtp.tile([128, KE, S], FP)           # 64 KB/part
        qT = big.tile([64, GH, S], FP)            # 32 KB/part on 64 parts
        kvT = big.tile([128, S], FP)              # 8 KB/part
        v_aug = big.tile([128, NT, HD + 2], FP)   # ones | v | ones
        ubarT = big.tile([64, GH, S], FP)         # 32 KB/part on 64 parts

        # ---- phase 1: load x, transpose to xT ----
        for i in range(S // 128):
            x_sb = xload.tile([128, E], FP)
            nc.sync.dma_start(out=x_sb, in_=x[bass.ts(i, 128), :])
            for jb in range(KE // 4):
                pt = rot.tile([128, 512], FP, tag="rot")
                for jj in range(4):
                    j = jb * 4 + jj
                    nc.tensor.transpose(
                        pt[:, bass.ts(jj, 128)], x_sb[:, bass.ts(j, 128)], ident
                    )
                nc.vector.tensor_copy(
                    out=mm(xT[:, bass.ds(jb * 4, 4), bass.ts(i, 128)]),
                    in_=pt.rearrange("p (a b) -> p a b", b=128),
                )

        # ---- phase 2: projections ----
        for sc in range(NSC):
            ssl = bass.ts(sc, SC)
            for h in range(GH):
                pq = rot.tile([128, 512], FP, tag="rot")
                for j in range(KE):
                    nc.tensor.matmul(
                        pq[0:HD, :],
                        mm(wq_sb[:, j, bass.ts(h, HD)]),
                        mm(xT[:, j, ssl]),
                        start=(j == 0),
                        stop=(j == KE - 1),
                    )
                nc.vector.tensor_scalar_add(
                    out=mm(qT[:, h, ssl]), in0=pq[0:HD, :], scalar1=bq_sb[:, h : h + 1]
                )
            pkv = rot.tile([128, 512], FP, tag="rot")
            for j in range(KE):
                nc.tensor.matmul(
                    pkv,
                    mm(wkv_sb[:, j, :]),
                    mm(xT[:, j, ssl]),
                    start=(j == 0),
                    stop=(j == KE - 1),
                )
            nc.vector.tensor_scalar_add(
                out=mm(kvT[:, ssl]), in0=pkv, scalar1=bkv_sb[:, 0:1]
            )

        # xT dead: release its pool so later pools reuse the space
        xtp_cm.__exit__(None, None, None)
        esb_pool = ctx.enter_context(tc.tile_pool(name="esb", bufs=4))
        zpool = ctx.enter_context(tc.tile_pool(name="zpool", bufs=2))

        # ---- phase 2b: v_aug = transpose(vT), ones columns both ends ----
        ones_v = ones_f[:, 0:NT].rearrange("p (a b) -> p a b", b=1)
        nc.vector.tensor_copy(out=mm(v_aug[:, :, 0:1]), in_=ones_v)
        nc.vector.tensor_copy(out=mm(v_aug[:, :, HD + 1 : HD + 2]), in_=ones_v)
        for ib in range(NT // 8):
            pt = rot.tile([128, 512], FP, tag="rot")
            for ii in range(8):
                i = ib * 8 + ii
                nc.tensor.transpose(
                    pt[:, bass.ts(ii, 64)],
                    kvT[HD : 2 * HD, bass.ts(i, 128)],
                    ident[HD : 2 * HD, HD : 2 * HD],
                )
            nc.vector.tensor_copy(
                out=mm(v_aug[:, bass.ds(ib * 8, 8), 1 : HD + 1]),
                in_=pt.rearrange("p (a b) -> p a b", b=HD),
            )

        # ---- phase 3: attention per (head, s-half) ----
        # All heads write A@V to PSUM base 0 (fp32r matmuls require dst
        # base partition 0): U rows 0:63, Z row 64 via the ones column.
        for h in range(GH):
            for sh in range(NSH):
                pav = pavp.tile([128, SH], FP, tag="pav")
                for t in range(NT):
                    psc = pscp.tile([128, SH], FP, tag="psc")
                    for u in range(SH // SC):
                        nc.tensor.matmul(
                            psc[:, bass.ts(u, SC)],
                            mm(kvT[0:HD, bass.ts(t, 128)]),
                            mm(qT[:, h, bass.ds(sh * SH + u * SC, SC)]),
                            start=True,
                            stop=True,
                        )
                    esb = esb_pool.tile([128, SH], FP, tag="esb")
                    nc.scalar.activation(
                        out=mm(esb), in_=psc,
                        func=mybir.ActivationFunctionType.Exp,
                        scale=1.0 / np.sqrt(HD),
                    )
                    for u in range(SH // SC):
                        nc.tensor.matmul(
                            pav[0 : HD + 1, bass.ts(u, SC)],
                            mm(v_aug[:, t, 1 : HD + 2]),
                            mm(esb[:, bass.ts(u, SC)]),
                            start=(t == 0),
                            stop=(t == NT - 1),
                        )
                # stage U rows, compute 1/Z, scale -- all at base 0
                shsl = bass.ds(sh * SH, SH)
                nc.vector.tensor_copy(
                    out=mm(ubarT[:, h, shsl]), in_=pav[0:HD, :]
                )
                zc = zpool.tile([128, SH], FP, tag="zc")
                nc.vector.tensor_copy(
                    out=zc[HD : HD + 1, :], in_=pav[HD : HD + 1, :]
                )
                nc.vector.reciprocal(zc[HD : HD + 1, :], zc[HD : HD + 1, :])
                zrr = zpool.tile([128, SH], FP, tag="zrr")
                nc.vector.tensor_copy(
                    out=mm(zrr[HD : HD + 1, :]), in_=zc[HD : HD + 1, :]
                )
                for u in range(SH // SC):
                    zbt = rot.tile([128, 512], FP, tag="rot")
                    nc.tensor.matmul(
                        zbt[0:HD, :],
                        mm(ones_col[HD : HD + 1, :]),
                        mm(zrr[HD : HD + 1, bass.ts(u, SC)]),
                        start=True,
                        stop=True,
                    )
                    usl = bass.ds(sh * SH + u * SC, SC)
                    nc.vector.tensor_mul(
                        out=mm(ubarT[:, h, usl]),
                        in0=ubarT[:, h, usl],
                        in1=zbt[0:HD, :],
                    )

        # ---- phase 4: output projection (DMA cannot read PSUM: stage) ----
        for sc in range(NSC):
            ssl = bass.ts(sc, SC)
            for et in range(KE):
                po = rot.tile([128, 512], FP, tag="rot")
                for c in range(GH):
                    nc.tensor.matmul(
                        po,
                        mm(wo_sb[:, c, bass.ts(et, 128)]),
                        mm(ubarT[:, c, ssl]),
                        start=(c == 0),
                        stop=(c == GH - 1),
                    )
                ost = xload.tile([128, 512], FP, tag="ost")
                nc.vector.tensor_copy(out=ost, in_=po)
                nc.sync.dma_start(out=ot[bass.ts(et, 128), ssl], in_=ost)

    nc.compile()
    return nc


_prog_cache: dict[str, bass.Bass] = {}


def kernel(x, Wq, bq, Wk, bk, Wv, bv, Wo, bo):
    x = np.ascontiguousarray(np.asarray(x, dtype=np.float32))
    Wq = np.asarray(Wq, dtype=np.float32)
    Wk = np.asarray(Wk, dtype=np.float32)
    Wv = np.asarray(Wv, dtype=np.float32)
    Wo = np.asarray(Wo, dtype=np.float32)
    bq = np.asarray(bq, dtype=np.float32)
    bk = np.asarray(bk, dtype=np.float32)
    bv = np.asarray(bv, dtype=np.float32)
    bo = np.asarray(bo, dtype=np.float32)

    if "nc" not in _prog_cache:
        _prog_cache["nc"] = build_program()
    nc = _prog_cache["nc"]

    in_maps = []
    for c in range(N_CORES):
        b, g = c // G, c % G
        in_maps.append(
            {
                "xc": np.ascontiguousarray(x[b]),
                "wq": np.ascontiguousarray(Wq[:, g * DG : (g + 1) * DG]),
                "wkv": np.ascontiguousarray(
                    np.concatenate(
                        [Wk[:, g * HD : (g + 1) * HD], Wv[:, g * HD : (g + 1) * HD]],
                        axis=1,
                    )
                ),
                "wo": np.ascontiguousarray(Wo[g * DG : (g + 1) * DG, :]),
                "bq": np.ascontiguousarray(bq[g * DG : (g + 1) * DG]),
                "bkv": np.ascontiguousarray(
                    np.concatenate(
                        [bk[g * HD : (g + 1) * HD], bv[g * HD : (g + 1) * HD]]
                    )
                ),
            }
        )

    global _last_in_maps
    _last_in_maps = in_maps
    res = run_bass_kernel_spmd(nc, in_maps, list(range(N_CORES))).results

    out = np.empty((B, S, E), dtype=np.float32)
    for b in range(B):
        acc = res[b * G]["ot"].astype(np.float32)
        for g in range(1, G):
            acc = acc + res[b * G + g]["ot"]
        out[b] = acc.T + bo
    return out



# revision 18
# speedup vs baseline: 3248.2515x; 3248.2515x over previous
"""GroupQueryAttention on 8 trn2 cores (bf16 compute, fp32 accumulate).

Sharding: core c = (b, g) with b = c // 4 (batch), g = c % 4 (KV group).
Each core computes the 4 query heads of its group against its batch's
sequence plus the row-slice of the output projection for those heads.
Host sums the 4 partial outputs per batch (row-parallel Wo) and adds bo.

All matmul operands are bf16 (psum accumulation fp32): same PE streaming
rate as fp32r at N=512 but none of fp32r's pre-rounding/base-partition
constraints, and half the host->device transfer bytes.  x arrives
pre-transposed ([E, S]) and weights pre-tiled ([128, chunk, cols]) so
every input DMA is a fully-contiguous burst and the PE does no transposes
except the small V retile.

Per-core schedule:
  xT   [e=128 x 8, s=2048] bf16   DMA'd directly (host transposed)
  qT   [d=64, h=4, s=2048] bf16   = Wq_g^T x^T + bq, M=128 packed matmuls
  kvT  [128, s=2048] bf16         rows 0:64 k^T, 64:128 v^T (+bkv)
  v_aug[t=128 x 16, 65] bf16      v re-transposed (PE), col 64 = 1.0
  per (s-half, head): for t in 16 tiles:
      scoresT psum [t=128, 1024] = k^T(tile)^T @ q^T   (2 matmuls)
      E = exp(0.125 * scoresT) -> bf16                 (1 ACT op per tile)
      U^T psum [65, 1024] += v_aug(t)^T @ E            (row 64 = Z)
    The A@V matmuls for tile t are emitted after the scores matmuls for
    tile t+1 so exp(t) overlaps scores(t+1) and the PE never waits on ACT.
  normalize (no PE): U|Z -> SBUF, 1/Z at partition 0 (DVE), broadcast to
    64 partitions with gpsimd partition_broadcast (Pool), one fused DVE
    multiply writes ubT pairs (odd heads partition-shifted to 64:128)
  out psum [s=128, e=1024] = (ubT pair)^T @ Wo rows, K=128 packed
      -> bf16 (DVE/ACT alternating) -> DMA to ot[S, E]
  host: out[b] = sum_g ot_g + bo
"""

import numpy as np
from contextlib import ExitStack

import ml_dtypes

import concourse.bass as bass
import concourse.bacc as bacc
import concourse.mybir as mybir
from concourse.tile import TileContext
from concourse.bass_utils import run_bass_kernel_spmd
B, S, E = 2, 2048, 1024
H, G, HD = 16, 4, 64
GH = H // G          # heads per group = 4
DG = GH * HD         # q cols per group = 256
N_CORES = 8

FP = mybir.dt.float32
BF = mybir.dt.bfloat16
BF_NP = ml_dtypes.bfloat16

KE = E // 128        # 8 contraction chunks for projections
NT = S // 128        # 16 t tiles
SC = 512             # matmul moving-dim chunk
NSC = S // SC        # 4
SH = 1024            # s-half for attention psum accumulators
NSH = S // SH        # 2


def build_program(loop_n: int = 1, upto: int = 4) -> bass.Bass:
    # Bacc (not plain Bass): its compile() runs move_matmul_waits_to_ldweights
    # + generate_event_semaphores, without which walrus rejects matmuls that
    # accumulated >1 semaphore wait ("Too many sync wait commands").
    nc = bacc.Bacc(None, target_bir_lowering=False)
    xt = nc.dram_tensor("xt", [E, S], BF, kind="ExternalInput")
    wq = nc.dram_tensor("wq", [128, KE, DG], BF, kind="ExternalInput")
    wkv = nc.dram_tensor("wkv", [128, KE, 2 * HD], BF, kind="ExternalInput")
    wo = nc.dram_tensor("wo", [128, DG // 128, E], BF, kind="ExternalInput")
    bq = nc.dram_tensor("bq", [DG], FP, kind="ExternalInput")
    bkv = nc.dram_tensor("bkv", [2 * HD], FP, kind="ExternalInput")
    ot = nc.dram_tensor("ot", [S, E], BF, kind="ExternalOutput")

    with TileContext(nc) as tc, ExitStack() as ctx:
        const = ctx.enter_context(tc.tile_pool(name="const", bufs=1))
        big = ctx.enter_context(tc.tile_pool(name="big", bufs=1))
        zpool = ctx.enter_context(tc.tile_pool(name="zpool", bufs=2))
        outp = ctx.enter_context(tc.tile_pool(name="outp", bufs=3))
        # PSUM banks: psc 3x2 + pav 2 = 8 of 8
        pscp = ctx.enter_context(tc.tile_pool(name="pscp", bufs=3, space="PSUM"))
        pavp = ctx.enter_context(tc.tile_pool(name="pavp", bufs=1, space="PSUM"))

        # ---- constants (outside any repeat loop) ----
        ones_bf = const.tile([128, 128], BF)
        nc.vector.memset(ones_bf, 1.0)
        bv_row = const.tile([1, HD], BF)

        wq_sb = const.tile([128, KE, DG], BF)
        wkv_sb = const.tile([128, KE, 2 * HD], BF)
        wo_sb = const.tile([128, DG // 128, E], BF)
        bq_sb = const.tile([128, DG // 128], FP)
        bkv_sb = const.tile([128, 1], FP)

        # ---- persistent activations ----
        xT = big.tile([128, NSC, KE, SC], BF)     # 32 KB/part, chunk-major
        qT = big.tile([128, DG // 128, S], BF)    # head pairs stacked
        kvT = big.tile([128, S], BF)              # 4 KB
        v_aug = big.tile([128, NT, HD + 1], BF)   # v | ones
        esb_ring = big.tile([128, 6, SH], BF)     # manual exp-output ring
        ubT = big.tile([128, DG // 128, S], BF)   # head pairs stacked

        def emit_body():
            xt_r = xt.rearrange("(j p) s -> p j s", p=128)

            # PE pstate warmup: dep-free tiny matmuls keep the tensor engine
            # continuously busy through the initial DMA window so the first
            # projection matmuls run at full clock (ramp needs ~3us busy).
            if upto >= 2:
                wup = pscp.tile([128, SH], FP, tag="psc")
                for _ in range(250):
                    nc.tensor.matmul(
                        wup[0:NT, 0:NT],
                        ones_bf[:, 0:NT],
                        ones_bf[:, 0:NT],
                        start=True,
                        stop=True,
                    )
                # dummy reader keeps the verifier happy; ubT is fully
                # overwritten by the normalize muls before phase 4 reads it
                nc.vector.tensor_copy(out=ubT[0:NT, 0, 0:NT], in_=wup[0:NT, 0:NT])

            # ---- phases 1+2 interleaved per 512-wide s-chunk:
            # DMA x^T chunk, then project it (q packed M=128: 2 heads/matmul)
            for sc in range(NSC):
                ssl = bass.ts(sc, SC)
                nc.sync.dma_start(out=xT[:, sc, :, :], in_=xt_r[:, :, ssl])
                if sc == 0:
                    nc.sync.dma_start(out=wq_sb, in_=wq[:, :, :])
                    nc.sync.dma_start(
                        out=bq_sb, in_=bq.rearrange("(j p) -> p j", p=128)
                    )
                    nc.sync.dma_start(out=wkv_sb, in_=wkv[:, :, :])
                    nc.sync.dma_start(
                        out=bkv_sb, in_=bkv.rearrange("(j p) -> p j", p=128)
                    )
                    bvt = zpool.tile([1, HD], FP, tag="bvt")
                    nc.sync.dma_start(
                        out=bvt, in_=bkv.rearrange("(j d) -> j d", j=2)[1:2, :]
                    )
                    nc.vector.tensor_copy(out=bv_row, in_=bvt)
                elif sc == 1:
                    nc.sync.dma_start(out=wo_sb, in_=wo[:, :, :])
                if upto < 2:
                    continue
                for m in range(DG // 128):
                    pq = pscp.tile([128, SH], FP, tag="psc")
                    for j in range(KE):
                        nc.tensor.matmul(
                            pq[:, 0:SC],
                            wq_sb[:, j, bass.ts(m, 128)],
                            xT[:, sc, j, :],
                            start=(j == 0),
                            stop=(j == KE - 1),
                        )
                    if m == 0:
                        nc.vector.tensor_scalar_add(
                            out=qT[:, m, ssl],
                            in0=pq[:, 0:SC],
                            scalar1=bq_sb[:, m : m + 1],
                        )
                    else:
                        nc.scalar.add(
                            out=qT[:, m, ssl], in_=pq[:, 0:SC],
                            add=bq_sb[:, m : m + 1],
                        )
                pkv = pscp.tile([128, SH], FP, tag="psc")
                for j in range(KE):
                    nc.tensor.matmul(
                        pkv[0:HD, 0:SC],
                        wkv_sb[:, j, 0:HD],
                        xT[:, sc, j, :],
                        start=(j == 0),
                        stop=(j == KE - 1),
                    )
                # k^T duplicated to both partition halves so scores can use
                # matching partition offsets for odd heads (qT pair layout)
                nc.vector.tensor_scalar_add(
                    out=kvT[0:HD, ssl],
                    in0=pkv[0:HD, 0:SC],
                    scalar1=bkv_sb[0:HD, 0:1],
                )
                nc.scalar.add(
                    out=kvT[HD : 2 * HD, ssl], in_=pkv[0:HD, 0:SC],
                    add=bkv_sb[0:HD, 0:1],
                )
                # v directly in [s, d] layout for the A@V stationary:
                # out[s, d] = x^T(chunk)^T @ Wv + 1 (x) bv, 4 chunks per sc
                pv = pscp.tile([128, SH], FP, tag="psc")
                for cc in range(SC // 128):
                    ci = sc * (SC // 128) + cc
                    for j in range(KE):
                        nc.tensor.matmul(
                            pv[:, bass.ds(cc * HD, HD)],
                            xT[:, sc, j, bass.ts(cc, 128)],
                            wkv_sb[:, j, HD : 2 * HD],
                            start=(j == 0),
                            stop=False,
                        )
                    nc.tensor.matmul(
                        pv[:, bass.ds(cc * HD, HD)],
                        ones_bf[0:1, :],
                        bv_row[:, :],
                        start=False,
                        stop=True,
                    )
                nc.vector.tensor_copy(
                    out=v_aug[:, bass.ds(sc * (SC // 128), SC // 128), 0:HD],
                    in_=pv[:, 0 : (SC // 128) * HD].rearrange(
                        "p (a b) -> p a b", b=HD
                    ),
                )

            if upto < 3:
                return

            # ones column of v_aug (Z row of the A@V accumulator)
            ones_v = ones_bf[:, 0:NT].rearrange("p (a b) -> p a b", b=1)
            nc.vector.tensor_copy(out=v_aug[:, :, HD : HD + 1], in_=ones_v)

            # ---- phase 3: attention per (s-half, head) ----
            for sh in range(NSH):
                for h in range(GH):
                    pav = pavp.tile([HD + 1, SH], FP, tag="pav")
                    esbs = {}
                    DEPTH = 2  # A@V for tile t issues after scores(t+DEPTH)
                    for t in range(NT + DEPTH):
                        if t < NT:
                            psc = pscp.tile([128, SH], FP, tag="psc")
                            po2 = (h % 2) * HD
                            for u in range(SH // SC):
                                nc.tensor.matmul(
                                    psc[:, bass.ts(u, SC)],
                                    kvT[po2 : po2 + HD, bass.ts(t, 128)],
                                    qT[po2 : po2 + HD, h // 2,
                                       bass.ds(sh * SH + u * SC, SC)],
                                    start=True,
                                    stop=True,
                                )
                            esb = esb_ring[:, t % 6, :]
                            nc.scalar.activation(
                                out=esb, in_=psc,
                                func=mybir.ActivationFunctionType.Exp,
                                scale=1.0 / np.sqrt(HD),
                            )
                            esbs[t] = esb
                        ta = t - DEPTH
                        if ta >= 0:
                            esb_a = esbs.pop(ta)
                            for u in range(SH // SC):
                                nc.tensor.matmul(
                                    pav[:, bass.ts(u, SC)],
                                    v_aug[:, ta, :],
                                    esb_a[:, bass.ts(u, SC)],
                                    start=(ta == 0),
                                    stop=(ta == NT - 1),
                                )
                    # ---- normalize (no PE): U|Z -> SBUF, bcast 1/Z, mul ----
                    ustg = zpool.tile([HD + 1, SH], FP, tag="ustg")
                    nc.vector.tensor_copy(out=ustg, in_=pav)
                    zr = zpool.tile([1, SH], FP, tag="zr")
                    nc.vector.tensor_copy(out=zr, in_=ustg[HD : HD + 1, :])
                    nc.vector.reciprocal(zr, zr)
                    zbc = zpool.tile([HD, SH], FP, tag="zbc")
                    nc.gpsimd.partition_broadcast(zbc, zr)
                    po2 = (h % 2) * HD
                    pair = h // 2
                    nc.vector.tensor_mul(
                        out=ubT[po2 : po2 + HD, pair, bass.ts(sh, SH)],
                        in0=ustg[0:HD, :],
                        in1=zbc,
                    )

            if upto < 4:
                return

            # ---- phase 4: output projection, out[s, e] ----
            for st in range(S // 128):
                ssl = bass.ts(st, 128)
                po = pscp.tile([128, SH], FP, tag="psc")
                for e2 in range(E // SC):
                    for j in range(DG // 128):
                        nc.tensor.matmul(
                            po[:, bass.ts(e2, SC)],
                            ubT[:, j, ssl],
                            wo_sb[:, j, bass.ts(e2, SC)],
                            start=(j == 0),
                            stop=(j == DG // 128 - 1),
                        )
                ost = outp.tile([128, E], BF, tag="ost")
                if st % 2 == 0:
                    nc.vector.tensor_copy(out=ost, in_=po)
                else:
                    nc.scalar.copy(out=ost, in_=po)
                nc.sync.dma_start(out=ot[ssl, :], in_=ost)

        if loop_n == 1:
            emit_body()
        else:
            with tc.For_i(0, loop_n):
                emit_body()

    nc.compile()
    return nc


_prog_cache: dict[str, bass.Bass] = {}


def _in_maps(x, Wq, bq, Wk, bk, Wv, bv, Wo, bo):
    xf = np.asarray(x, dtype=np.float32)
    Wqb = np.asarray(Wq, dtype=np.float32).astype(BF_NP)
    Wkb = np.asarray(Wk, dtype=np.float32).astype(BF_NP)
    Wvb = np.asarray(Wv, dtype=np.float32).astype(BF_NP)
    Wob = np.asarray(Wo, dtype=np.float32).astype(BF_NP)
    bqf = np.asarray(bq, dtype=np.float32)
    bkf = np.asarray(bk, dtype=np.float32)
    bvf = np.asarray(bv, dtype=np.float32)

    def tile_rows(w):
        # [n*128, c] -> [128, n, c] with row j*128+p at [p, j]
        n = w.shape[0] // 128
        return np.ascontiguousarray(w.reshape(n, 128, -1).transpose(1, 0, 2))

    maps = []
    for c in range(N_CORES):
        b, g = c // G, c % G
        wkv_g = np.concatenate(
            [Wkb[:, g * HD : (g + 1) * HD], Wvb[:, g * HD : (g + 1) * HD]], axis=1
        )
        maps.append(
            {
                "xt": np.ascontiguousarray(xf[b].T).astype(BF_NP),
                "wq": tile_rows(Wqb[:, g * DG : (g + 1) * DG]),
                "wkv": tile_rows(wkv_g),
                "wo": tile_rows(Wob[g * DG : (g + 1) * DG, :]),
                "bq": np.ascontiguousarray(bqf[g * DG : (g + 1) * DG]),
                "bkv": np.ascontiguousarray(
                    np.concatenate(
                        [bkf[g * HD : (g + 1) * HD], bvf[g * HD : (g + 1) * HD]]
                    )
                ),
            }
        )
    return maps


def kernel(x, Wq, bq, Wk, bk, Wv, bv, Wo, bo):
    if "nc" not in _prog_cache:
        _prog_cache["nc"] = build_program()
    nc = _prog_cache["nc"]

    in_maps = _in_maps(x, Wq, bq, Wk, bk, Wv, bv, Wo, bo)
    global _last_in_maps
    _last_in_maps = in_maps
    res = run_bass_kernel_spmd(nc, in_maps, list(range(N_CORES))).results

    bo = np.asarray(bo, dtype=np.float32)
    out = np.empty((B, S, E), dtype=np.float32)
    for b in range(B):
        acc = res[b * G]["ot"].astype(np.float32)
        for g in range(1, G):
            acc = acc + res[b * G + g]["ot"].astype(np.float32)
        out[b] = acc + bo
    return out


# revision 21
# speedup vs baseline: 3580.6074x; 1.1023x over previous
"""GroupQueryAttention on 8 trn2 cores (bf16 compute, fp32 accumulate).

Sharding: core c = (b, g) with b = c // 4 (batch), g = c % 4 (KV group).
Each core computes the 4 query heads of its group against its batch's
sequence plus the row-slice of the output projection for those heads.
Host sums the 4 partial outputs per batch (row-parallel Wo) and adds bo.

All matmul operands are bf16 (psum accumulation fp32): same PE streaming
rate as fp32r at N=512 but none of fp32r's pre-rounding/base-partition
constraints, and half the host->device transfer bytes.  x arrives
pre-transposed ([E, S]) and weights pre-tiled ([128, chunk, cols]) so
every input DMA is a fully-contiguous burst and the PE does no transposes
except the small V retile.

Per-core schedule:
  xT   [e=128 x 8, s=2048] bf16   DMA'd directly (host transposed)
  qT   [d=64, h=4, s=2048] bf16   = Wq_g^T x^T + bq, M=128 packed matmuls
  kvT  [128, s=2048] bf16         rows 0:64 k^T, 64:128 v^T (+bkv)
  v_aug[t=128 x 16, 65] bf16      v re-transposed (PE), col 64 = 1.0
  per (s-half, head): for t in 16 tiles:
      scoresT psum [t=128, 1024] = k^T(tile)^T @ q^T   (2 matmuls)
      E = exp(0.125 * scoresT) -> bf16                 (1 ACT op per tile)
      U^T psum [65, 1024] += v_aug(t)^T @ E            (row 64 = Z)
    The A@V matmuls for tile t are emitted after the scores matmuls for
    tile t+3 (DEPTH=3 software pipeline) so exp(t) runs on ACT strictly
    under the PE's scores stream; phase 3 is ACT-exp-bound (~1.04us/tile).
  normalize (no PE): U|Z -> SBUF, 1/Z at partition 0 (DVE), broadcast to
    64 partitions with gpsimd partition_broadcast (Pool), one fused DVE
    multiply writes ubT pairs (odd heads partition-shifted to 64:128)
  out psum [s=128, e=1024] = (ubT pair)^T @ Wo rows, K=128 packed
      -> bf16 (DVE/ACT alternating) -> DMA to ot[S, E]
  host: out[b] = sum_g ot_g + bo
"""

import numpy as np
from contextlib import ExitStack

import ml_dtypes

import concourse.bass as bass
import concourse.bacc as bacc
import concourse.mybir as mybir
from concourse.tile import TileContext
from concourse.bass_utils import run_bass_kernel_spmd
B, S, E = 2, 2048, 1024
H, G, HD = 16, 4, 64
GH = H // G          # heads per group = 4
DG = GH * HD         # q cols per group = 256
N_CORES = 8

FP = mybir.dt.float32
BF = mybir.dt.bfloat16
BF_NP = ml_dtypes.bfloat16

KE = E // 128        # 8 contraction chunks for projections
NT = S // 128        # 16 t tiles
SC = 512             # matmul moving-dim chunk
NSC = S // SC        # 4
SH = 1024            # s-half for attention psum accumulators
NSH = S // SH        # 2


def build_program(loop_n: int = 1, upto: int = 4) -> bass.Bass:
    # Bacc (not plain Bass): its compile() runs move_matmul_waits_to_ldweights
    # + generate_event_semaphores, without which walrus rejects matmuls that
    # accumulated >1 semaphore wait ("Too many sync wait commands").
    nc = bacc.Bacc(None, target_bir_lowering=False)
    xt = nc.dram_tensor("xt", [E, S], BF, kind="ExternalInput")
    wq = nc.dram_tensor("wq", [128, KE, DG], BF, kind="ExternalInput")
    wkv = nc.dram_tensor("wkv", [128, KE, 2 * HD], BF, kind="ExternalInput")
    wo = nc.dram_tensor("wo", [128, DG // 128, E], BF, kind="ExternalInput")
    bq = nc.dram_tensor("bq", [DG], FP, kind="ExternalInput")
    bkv = nc.dram_tensor("bkv", [2 * HD], FP, kind="ExternalInput")
    ot = nc.dram_tensor("ot", [S, E], BF, kind="ExternalOutput")

    with TileContext(nc) as tc, ExitStack() as ctx:
        const = ctx.enter_context(tc.tile_pool(name="const", bufs=1))
        big = ctx.enter_context(tc.tile_pool(name="big", bufs=1))
        zpool = ctx.enter_context(tc.tile_pool(name="zpool", bufs=2))
        outp = ctx.enter_context(tc.tile_pool(name="outp", bufs=3))
        # PSUM banks: psc 3x2 + pav 2 = 8 of 8
        pscp = ctx.enter_context(tc.tile_pool(name="pscp", bufs=3, space="PSUM"))
        pavp = ctx.enter_context(tc.tile_pool(name="pavp", bufs=1, space="PSUM"))

        # ---- constants (outside any repeat loop) ----
        ones_bf = const.tile([128, 128], BF)
        nc.vector.memset(ones_bf, 1.0)
        bv_row = const.tile([1, HD], BF)

        wq_sb = const.tile([128, KE, DG], BF)
        wkv_sb = const.tile([128, KE, 2 * HD], BF)
        wo_sb = const.tile([128, DG // 128, E], BF)
        bq_sb = const.tile([128, DG // 128], FP)
        bkv_sb = const.tile([128, 1], FP)

        # ---- persistent activations ----
        xT = big.tile([128, NSC, KE, SC], BF)     # 32 KB/part, chunk-major
        qT = big.tile([128, DG // 128, S], BF)    # head pairs stacked
        kvT = big.tile([128, S], BF)              # 4 KB
        v_aug = big.tile([128, NT, HD + 1], BF)   # v | ones
        esb_ring = big.tile([128, 6, SH], BF)     # manual exp-output ring
        ubT = big.tile([128, DG // 128, S], BF)   # head pairs stacked

        def emit_body():
            xt_r = xt.rearrange("(j p) s -> p j s", p=128)

            # PE pstate warmup: dep-free tiny matmuls keep the tensor engine
            # continuously busy through the initial DMA window so the first
            # projection matmuls run at full clock (ramp needs ~3us busy).
            if upto >= 2:
                wup = pscp.tile([128, SH], FP, tag="psc")
                for _ in range(250):
                    nc.tensor.matmul(
                        wup[0:NT, 0:NT],
                        ones_bf[:, 0:NT],
                        ones_bf[:, 0:NT],
                        start=True,
                        stop=True,
                    )
                # dummy reader keeps the verifier happy; ubT is fully
                # overwritten by the normalize muls before phase 4 reads it
                nc.vector.tensor_copy(out=ubT[0:NT, 0, 0:NT], in_=wup[0:NT, 0:NT])

            # ---- phases 1+2 interleaved per 512-wide s-chunk:
            # DMA x^T chunk, then project it (q packed M=128: 2 heads/matmul)
            for sc in range(NSC):
                ssl = bass.ts(sc, SC)
                nc.sync.dma_start(out=xT[:, sc, :, :], in_=xt_r[:, :, ssl])
                if sc == 0:
                    nc.sync.dma_start(out=wq_sb, in_=wq[:, :, :])
                    nc.sync.dma_start(
                        out=bq_sb, in_=bq.rearrange("(j p) -> p j", p=128)
                    )
                    nc.sync.dma_start(out=wkv_sb, in_=wkv[:, :, :])
                    nc.sync.dma_start(
                        out=bkv_sb, in_=bkv.rearrange("(j p) -> p j", p=128)
                    )
                    bvt = zpool.tile([1, HD], FP, tag="bvt")
                    nc.sync.dma_start(
                        out=bvt, in_=bkv.rearrange("(j d) -> j d", j=2)[1:2, :]
                    )
                    nc.vector.tensor_copy(out=bv_row, in_=bvt)
                elif sc == 1:
                    nc.sync.dma_start(out=wo_sb, in_=wo[:, :, :])
                if upto < 2:
                    continue
                for m in range(DG // 128):
                    pq = pscp.tile([128, SH], FP, tag="psc")
                    for j in range(KE):
                        nc.tensor.matmul(
                            pq[:, 0:SC],
                            wq_sb[:, j, bass.ts(m, 128)],
                            xT[:, sc, j, :],
                            start=(j == 0),
                            stop=(j == KE - 1),
                        )
                    if m == 0:
                        nc.vector.tensor_scalar_add(
                            out=qT[:, m, ssl],
                            in0=pq[:, 0:SC],
                            scalar1=bq_sb[:, m : m + 1],
                        )
                    else:
                        nc.scalar.add(
                            out=qT[:, m, ssl], in_=pq[:, 0:SC],
                            add=bq_sb[:, m : m + 1],
                        )
                pkv = pscp.tile([128, SH], FP, tag="psc")
                for j in range(KE):
                    nc.tensor.matmul(
                        pkv[0:HD, 0:SC],
                        wkv_sb[:, j, 0:HD],
                        xT[:, sc, j, :],
                        start=(j == 0),
                        stop=(j == KE - 1),
                    )
                # k^T duplicated to both partition halves so scores can use
                # matching partition offsets for odd heads (qT pair layout)
                nc.vector.tensor_scalar_add(
                    out=kvT[0:HD, ssl],
                    in0=pkv[0:HD, 0:SC],
                    scalar1=bkv_sb[0:HD, 0:1],
                )
                nc.scalar.add(
                    out=kvT[HD : 2 * HD, ssl], in_=pkv[0:HD, 0:SC],
                    add=bkv_sb[0:HD, 0:1],
                )
                # v directly in [s, d] layout for the A@V stationary:
                # out[s, d] = x^T(chunk)^T @ Wv + 1 (x) bv, 4 chunks per sc
                pv = pscp.tile([128, SH], FP, tag="psc")
                for cc in range(SC // 128):
                    ci = sc * (SC // 128) + cc
                    for j in range(KE):
                        nc.tensor.matmul(
                            pv[:, bass.ds(cc * HD, HD)],
                            xT[:, sc, j, bass.ts(cc, 128)],
                            wkv_sb[:, j, HD : 2 * HD],
                            start=(j == 0),
                            stop=False,
                        )
                    nc.tensor.matmul(
                        pv[:, bass.ds(cc * HD, HD)],
                        ones_bf[0:1, :],
                        bv_row[:, :],
                        start=False,
                        stop=True,
                    )
                nc.vector.tensor_copy(
                    out=v_aug[:, bass.ds(sc * (SC // 128), SC // 128), 0:HD],
                    in_=pv[:, 0 : (SC // 128) * HD].rearrange(
                        "p (a b) -> p a b", b=HD
                    ),
                )

            if upto < 3:
                return

            # ones column of v_aug (Z row of the A@V accumulator)
            ones_v = ones_bf[:, 0:NT].rearrange("p (a b) -> p a b", b=1)
            nc.vector.tensor_copy(out=v_aug[:, :, HD : HD + 1], in_=ones_v)

            # ---- phase 3: attention per (s-half, head) ----
            for sh in range(NSH):
                for h in range(GH):
                    pav = pavp.tile([HD + 1, SH], FP, tag="pav")
                    esbs = {}
                    DEPTH = 3  # A@V for tile t issues after scores(t+DEPTH)
                    for t in range(NT + DEPTH):
                        if t < NT:
                            psc = pscp.tile([128, SH], FP, tag="psc")
                            po2 = (h % 2) * HD
                            for u in range(SH // SC):
                                nc.tensor.matmul(
                                    psc[:, bass.ts(u, SC)],
                                    kvT[po2 : po2 + HD, bass.ts(t, 128)],
                                    qT[po2 : po2 + HD, h // 2,
                                       bass.ds(sh * SH + u * SC, SC)],
                                    start=True,
                                    stop=True,
                                )
                            esb = esb_ring[:, t % 6, :]
                            nc.scalar.activation(
                                out=esb, in_=psc,
                                func=mybir.ActivationFunctionType.Exp,
                                scale=1.0 / np.sqrt(HD),
                            )
                            esbs[t] = esb
                        ta = t - DEPTH
                        if ta >= 0:
                            esb_a = esbs.pop(ta)
                            for u in range(SH // SC):
                                nc.tensor.matmul(
                                    pav[:, bass.ts(u, SC)],
                                    v_aug[:, ta, :],
                                    esb_a[:, bass.ts(u, SC)],
                                    start=(ta == 0),
                                    stop=(ta == NT - 1),
                                )
                    # ---- normalize (no PE): U|Z -> SBUF, bcast 1/Z, mul ----
                    ustg = zpool.tile([HD + 1, SH], FP, tag="ustg")
                    nc.vector.tensor_copy(out=ustg, in_=pav)
                    zr = zpool.tile([1, SH], FP, tag="zr")
                    nc.vector.tensor_copy(out=zr, in_=ustg[HD : HD + 1, :])
                    nc.vector.reciprocal(zr, zr)
                    zbc = zpool.tile([HD, SH], FP, tag="zbc")
                    nc.gpsimd.partition_broadcast(zbc, zr)
                    po2 = (h % 2) * HD
                    pair = h // 2
                    nc.vector.tensor_mul(
                        out=ubT[po2 : po2 + HD, pair, bass.ts(sh, SH)],
                        in0=ustg[0:HD, :],
                        in1=zbc,
                    )

            if upto < 4:
                return

            # ---- phase 4: output projection, out[s, e] ----
            for st in range(S // 128):
                ssl = bass.ts(st, 128)
                po = pscp.tile([128, SH], FP, tag="psc")
                for e2 in range(E // SC):
                    for j in range(DG // 128):
                        nc.tensor.matmul(
                            po[:, bass.ts(e2, SC)],
                            ubT[:, j, ssl],
                            wo_sb[:, j, bass.ts(e2, SC)],
                            start=(j == 0),
                            stop=(j == DG // 128 - 1),
                        )
                ost = outp.tile([128, E], BF, tag="ost")
                if st % 2 == 0:
                    nc.vector.tensor_copy(out=ost, in_=po)
                else:
                    nc.scalar.copy(out=ost, in_=po)
                nc.sync.dma_start(out=ot[ssl, :], in_=ost)

        if loop_n == 1:
            emit_body()
        else:
            with tc.For_i(0, loop_n):
                emit_body()

    nc.compile()
    return nc


_prog_cache: dict[str, bass.Bass] = {}


def _in_maps(x, Wq, bq, Wk, bk, Wv, bv, Wo, bo):
    xf = np.asarray(x, dtype=np.float32)
    Wqb = np.asarray(Wq, dtype=np.float32).astype(BF_NP)
    Wkb = np.asarray(Wk, dtype=np.float32).astype(BF_NP)
    Wvb = np.asarray(Wv, dtype=np.float32).astype(BF_NP)
    Wob = np.asarray(Wo, dtype=np.float32).astype(BF_NP)
    bqf = np.asarray(bq, dtype=np.float32)
    bkf = np.asarray(bk, dtype=np.float32)
    bvf = np.asarray(bv, dtype=np.float32)

    def tile_rows(w):
        # [n*128, c] -> [128, n, c] with row j*128+p at [p, j]
        n = w.shape[0] // 128
        return np.ascontiguousarray(w.reshape(n, 128, -1).transpose(1, 0, 2))

    maps = []
    for c in range(N_CORES):
        b, g = c // G, c % G
        wkv_g = np.concatenate(
            [Wkb[:, g * HD : (g + 1) * HD], Wvb[:, g * HD : (g + 1) * HD]], axis=1
        )
        maps.append(
            {
                "xt": np.ascontiguousarray(xf[b].T).astype(BF_NP),
                "wq": tile_rows(Wqb[:, g * DG : (g + 1) * DG]),
                "wkv": tile_rows(wkv_g),
                "wo": tile_rows(Wob[g * DG : (g + 1) * DG, :]),
                "bq": np.ascontiguousarray(bqf[g * DG : (g + 1) * DG]),
                "bkv": np.ascontiguousarray(
                    np.concatenate(
                        [bkf[g * HD : (g + 1) * HD], bvf[g * HD : (g + 1) * HD]]
                    )
                ),
            }
        )
    return maps


def kernel(x, Wq, bq, Wk, bk, Wv, bv, Wo, bo):
    if "nc" not in _prog_cache:
        _prog_cache["nc"] = build_program()
    nc = _prog_cache["nc"]

    in_maps = _in_maps(x, Wq, bq, Wk, bk, Wv, bv, Wo, bo)
    global _last_in_maps
    _last_in_maps = in_maps
    res = run_bass_kernel_spmd(nc, in_maps, list(range(N_CORES))).results

    bo = np.asarray(bo, dtype=np.float32)
    out = np.empty((B, S, E), dtype=np.float32)
    for b in range(B):
        acc = res[b * G]["ot"].astype(np.float32)
        for g in range(1, G):
            acc = acc + res[b * G + g]["ot"].astype(np.float32)
        out[b] = acc + bo
    return out
